# revision 1
# baseline (speedup 1.0000x reference)
"""CAM module (DANet channel attention) Trainium2 kernel.

Full inputs -> shard batch over 8 cores (2 batches/core) -> single SPMD Bass
kernel (energy + softmax + algebraic BN stats + AllReduce + fused output pass)
-> gather full output.

Math notes:
  energy = q @ q^T per batch; softmax(max-e) == softmax(-e).
  BN stats computed without materializing att@q:
     S1[c] = sum_n M[c,n]   = (att @ qsum)[c]
     S2[c] = sum_n M[c,n]^2 = rowsum((att @ energy) * att)   [M M^T = att E att^T]
  Final: out = s*M + t + q  with s = gamma*bnw*rsd, t = bnb - mean_M*s,
  rsd = 1/sqrt(gamma^2*var_M + eps).

All matmul operands use float32r (fp32 with 12-bit mantissa, 1 cycle/row at
N>=256) -- measured end-to-end error ~3e-4.
"""
import os
import sys

if '/opt/trn_rl_repo' not in sys.path:
    sys.path.insert(0, '/opt/trn_rl_repo')

import numpy as np

import concourse.bass as bass
import concourse.bacc as bacc
import concourse.mybir as mybir
import concourse.tile as tile
from concourse import masks
from concourse.bass_utils import run_bass_kernel_spmd

F32 = mybir.dt.float32
F32R = mybir.dt.float32r

N_CORES = 8
B, C, H, W = 16, 128, 128, 128
HW_FULL = H * W
B_LOC = B // N_CORES
BN_EPS = 1e-5


def _emit_stt(nc, mybir, y_tile, c, mps, t_vec, q_tile, outl, b):
    cs = slice(c * 512, (c + 1) * 512)
    qcol = slice((c % 4) * 512, (c % 4 + 1) * 512)
    F32 = mybir.dt.float32
    nc.vector.scalar_tensor_tensor(
        out=y_tile[:, qcol], in0=mps[:], scalar=t_vec[:, 0:1],
        in1=q_tile[:, cs].bitcast(F32),
        op0=mybir.AluOpType.add, op1=mybir.AluOpType.add)
    if c % 4 == 3:
        g0 = (c - 3) * 512
        nc.sync.dma_start(out=outl[b, :, g0:g0 + 2048], in_=y_tile[:])


def build(hw=HW_FULL, n_cores=N_CORES, b_loc=B_LOC, use_collective=True, skip=()):
    nc = bacc.Bacc("TRN2", target_bir_lowering=False, debug=False,
                   num_devices=n_cores)

    xl = nc.dram_tensor("xl", [b_loc, C, hw], F32, kind="ExternalInput")
    gamma = nc.dram_tensor("gamma", [1, 1], F32, kind="ExternalInput")
    bnw = nc.dram_tensor("bnw", [C, 1], F32, kind="ExternalInput")
    bnb = nc.dram_tensor("bnb", [C, 1], F32, kind="ExternalInput")
    outl = nc.dram_tensor("out", [b_loc, C, hw], F32, kind="ExternalOutput")
    debug = bool(os.environ.get("KERNEL_DEBUG"))
    if debug:
        d_enr = nc.dram_tensor("d_enr", [C, 130], F32, kind="ExternalOutput")
        d_att = nc.dram_tensor("d_att", [C, C], F32, kind="ExternalOutput")
        d_attT = nc.dram_tensor("d_attT", [C, C], F32, kind="ExternalOutput")
        d_r = nc.dram_tensor("d_r", [C, 130], F32, kind="ExternalOutput")
        d_stats = nc.dram_tensor("d_stats", [C, 2], F32, kind="ExternalOutput")
        d_st = nc.dram_tensor("d_st", [C, 2], F32, kind="ExternalOutput")

    n_slices = max(1, hw // 2048)          # DMA/rounding slices
    slice_w = hw // n_slices
    n_groups = hw // 512                   # transpose/matmul groups of 4 chunks
    n_chunks2 = hw // 512                  # pass-2 chunks
    n_total = float(n_cores * b_loc * hw)  # BN sample count per channel

    with tile.TileContext(nc) as tc:
        import contextlib
        ctx = contextlib.ExitStack()
        with ctx:
            singles = ctx.enter_context(tc.tile_pool(name="singles", bufs=1))
            pp_t = ctx.enter_context(tc.tile_pool(name="pp_t", bufs=2, space="PSUM"))
            pp_e = ctx.enter_context(tc.tile_pool(name="pp_e", bufs=2, space="PSUM"))
            pp_s = ctx.enter_context(tc.tile_pool(name="pp_s", bufs=1, space="PSUM"))
            pp_m = ctx.enter_context(tc.tile_pool(name="pp_m", bufs=2, space="PSUM"))
            sm = ctx.enter_context(tc.tile_pool(name="sm", bufs=2))
            pz = ctx.enter_context(tc.tile_pool(name="pz", bufs=3))
            pb = ctx.enter_context(tc.tile_pool(name="pb", bufs=3))
            py = ctx.enter_context(tc.tile_pool(name="py", bufs=3))
            dram = ctx.enter_context(tc.tile_pool(name="dram", bufs=1, space="DRAM"))

            # ---------------- setup ----------------
            ident = singles.tile([128, 128], F32)
            masks.make_identity(nc, ident[:])
            ident_r = singles.tile([128, 128], F32R)
            nc.vector.tensor_copy(ident_r[:], ident[:])

            # staging tiles: per slot [qT(128) | ones(1) | zeros(127)]
            z256 = singles.tile([128, 256], F32)
            nc.vector.memset(z256[:], 0.0)
            nc.vector.memset(z256[:, 128:129], 1.0)
            stages = []
            for i in range(3):
                st = singles.tile([128, 4, 256], F32R, name=f"stage{i}", tag=f"stage{i}")
                for j in range(4):
                    nc.vector.tensor_copy(st[:, j, :], z256[:])
                stages.append(st)

            bnw_sb = singles.tile([128, 1], F32)
            bnb_sb = singles.tile([128, 1], F32)
            nc.sync.dma_start(out=bnw_sb[:], in_=bnw[:, :])
            nc.sync.dma_start(out=bnb_sb[:], in_=bnb[:, :])
            gamma_sb = singles.tile([128, 1], F32)
            g_bcast = bass.AP(tensor=gamma.ap().tensor, offset=0, ap=[[0, 128], [1, 1]])
            nc.sync.dma_start(out=gamma_sb[:], in_=g_bcast)

            q_t = [singles.tile([128, hw], F32R, name=f"q{b}", tag=f"q{b}") for b in range(b_loc)]
            att_rs = [None] * b_loc
            attT_s = [singles.tile([128, 128], F32R, name=f"attTs{b}", tag=f"attTs{b}") for b in range(b_loc)]
            attT_r = [singles.tile([128, 128], F32R, name=f"attT{b}", tag=f"attT{b}") for b in range(b_loc)]
            stats_b = [singles.tile([128, 2], F32, name=f"stats{b}", tag=f"stats{b}") for b in range(b_loc)]

            # ---------------- phase 1: energy + softmax + stats ----------------
            for b in range(b_loc):
                for sl in range(n_slices):
                    cs = slice(sl * slice_w, (sl + 1) * slice_w)
                    bt = pb.tile([128, slice_w], F32, tag="bounce")
                    nc.sync.dma_start(out=bt[:], in_=xl[b, :, cs])
                    # round fp32 -> f32r while copying into the resident q tile
                    nc.scalar.copy(q_t[b][:, cs], bt[:])

                epsum = pp_e.tile([128, 256], F32, tag="epsum")

                def emit_tp(g):
                    psT = pp_t.tile([128, 4, 128], F32R, tag="psT", name="psT")
                    for j in range(4):
                        k = 4 * g + j
                        nc.tensor.transpose(
                            psT[:, j, :],
                            q_t[b][:, k * 128:(k + 1) * 128],
                            ident_r[:])
                    stage = stages[g % 3]
                    nc.vector.tensor_copy(stage[:, :, 0:128], psT[:].bitcast(F32))
                    return stage

                def emit_mm(g, stage):
                    if "mm" in skip:
                        return
                    for j in range(4):
                        nc.tensor.matmul(
                            epsum[:],
                            lhsT=stage[:, j, 0:128],
                            rhs=stage[:, j, :],
                            start=(g == 0 and j == 0),
                            stop=(g == n_groups - 1 and j == 3))

                if "tp" not in skip:
                    # software pipeline: transposes run one group ahead of matmuls
                    prev = None
                    for g in range(n_groups):
                        stage = emit_tp(g)
                        if prev is not None:
                            emit_mm(g - 1, prev)
                        prev = stage
                    emit_mm(n_groups - 1, prev)

                if "sm" in skip:
                    continue
                enr = sm.tile([128, 129], F32, tag="enr")
                nc.vector.tensor_copy(enr[:], epsum[:, 0:129])
                # 130 cols: f32r matmul needs an even moving-dim count
                enr_r = sm.tile([128, 130], F32R, tag="enr_r")
                nc.vector.tensor_copy(enr_r[:], epsum[:, 0:130])
                mrow = sm.tile([128, 1], F32, tag="mrow")
                nc.vector.tensor_reduce(out=mrow[:], in_=enr[:, 0:128],
                                        axis=mybir.AxisListType.X,
                                        op=mybir.AluOpType.min)
                p_sb = sm.tile([128, 128], F32, tag="p_sb")
                rs = sm.tile([128, 1], F32, tag="rs")
                nc.scalar.activation(out=p_sb[:], in_=enr[:, 0:128],
                                     func=mybir.ActivationFunctionType.Exp,
                                     bias=mrow[:, 0:1], scale=-1.0,
                                     accum_out=rs[:, 0:1])
                rinv = sm.tile([128, 1], F32, tag="rinv")
                nc.vector.reciprocal(rinv[:], rs[:])
                att_r = sm.tile([128, 128], F32R, tag="att_r",
                                name=f"att_r{b}")
                nc.vector.tensor_scalar_mul(out=att_r[:], in0=p_sb[:],
                                            scalar1=rinv[:, 0:1])
                attT_ps = pp_s.tile([128, 128], F32R, tag="attT_ps")
                nc.tensor.transpose(attT_ps[:], att_r[:], ident_r[:])
                nc.vector.tensor_copy(attT_r[b][:], attT_ps[:].bitcast(F32))

                r_ps = pp_s.tile([128, 130], F32, tag="r_ps")
                nc.tensor.matmul(r_ps[:], lhsT=attT_r[b][:], rhs=enr_r[:],
                                 start=True, stop=True)
                scr = sm.tile([128, 128], F32, tag="scr")
                nc.vector.tensor_mul(scr[:], r_ps[:, 0:128], att_r[:].bitcast(F32))
                nc.vector.tensor_reduce(out=stats_b[b][:, 1:2], in_=scr[:],
                                        axis=mybir.AxisListType.X,
                                        op=mybir.AluOpType.add)
                nc.vector.tensor_copy(stats_b[b][:, 0:1], r_ps[:, 128:129])
                att_rs[b] = att_r
                if debug and b == 0:
                    nc.sync.dma_start(out=d_enr[:, :], in_=enr_r[:].bitcast(F32))
                    nc.sync.dma_start(out=d_att[:, :], in_=att_r[:].bitcast(F32))
                    nc.sync.dma_start(out=d_attT[:, :], in_=attT_r[0][:].bitcast(F32))
                    rcp = sm.tile([128, 130], F32, tag="rcp")
                    nc.vector.tensor_copy(rcp[:], r_ps[:])
                    nc.sync.dma_start(out=d_r[:, :], in_=rcp[:])
                    nc.sync.dma_start(out=d_stats[:, :], in_=stats_b[0][:])

            # ---------------- stats reduce + s,t ----------------
            if "sm" in skip:
                for b in range(b_loc):
                    nc.vector.memset(stats_b[b][:], 1.0)
            stats_tot = singles.tile([128, 2], F32)
            nc.vector.tensor_add(stats_tot[:], stats_b[0][:], stats_b[1][:]) \
                if b_loc == 2 else nc.vector.tensor_copy(stats_tot[:], stats_b[0][:])

            if use_collective:
                cc_in = dram.tile([128, 2], F32)
                cc_out = dram.tile([128, 2], F32)
                nc.gpsimd.dma_start(out=cc_in[:], in_=stats_tot[:])
                nc.gpsimd.collective_compute(
                    "AllReduce", mybir.AluOpType.add,
                    replica_groups=[list(range(n_cores))],
                    ins=[cc_in.opt()], outs=[cc_out.opt()])
                stats_g = singles.tile([128, 2], F32)
                nc.gpsimd.dma_start(out=stats_g[:], in_=cc_out[:])
            else:
                stats_g = stats_tot

            inv_n = (1.0 / n_total) if use_collective else (1.0 / (b_loc * hw))
            meanM = singles.tile([128, 1], F32)
            nc.vector.tensor_scalar_mul(out=meanM[:], in0=stats_g[:, 0:1], scalar1=inv_n)
            em2 = singles.tile([128, 1], F32)
            nc.vector.tensor_scalar_mul(out=em2[:], in0=stats_g[:, 1:2], scalar1=inv_n)
            varM = singles.tile([128, 1], F32)
            nc.vector.tensor_mul(varM[:], meanM[:], meanM[:])
            nc.vector.tensor_sub(varM[:], em2[:], varM[:])
            # var_out = gamma^2 * varM ; sd = sqrt(var_out + eps); rsd = 1/sd
            nc.vector.tensor_mul(varM[:], varM[:], gamma_sb[:])
            nc.vector.tensor_mul(varM[:], varM[:], gamma_sb[:])
            eps_sb = singles.tile([128, 1], F32)
            nc.vector.memset(eps_sb[:], BN_EPS)
            sd = singles.tile([128, 1], F32)
            nc.scalar.activation(out=sd[:], in_=varM[:],
                                 func=mybir.ActivationFunctionType.Sqrt,
                                 bias=eps_sb[:, 0:1], scale=1.0)
            rsd = singles.tile([128, 1], F32)
            nc.vector.reciprocal(rsd[:], sd[:])
            s_vec = singles.tile([128, 1], F32)
            nc.vector.tensor_mul(s_vec[:], gamma_sb[:], bnw_sb[:])
            nc.vector.tensor_mul(s_vec[:], s_vec[:], rsd[:])
            t_vec = singles.tile([128, 1], F32)
            nc.vector.tensor_mul(t_vec[:], meanM[:], s_vec[:])
            nc.vector.tensor_sub(t_vec[:], bnb_sb[:], t_vec[:])
            if debug:
                stv = singles.tile([128, 2], F32)
                nc.vector.tensor_copy(stv[:, 0:1], s_vec[:])
                nc.vector.tensor_copy(stv[:, 1:2], t_vec[:])
                nc.sync.dma_start(out=d_st[:, :], in_=stv[:])

            # fold s into the attention, re-transpose (pass 2 then needs no ACT)
            for b in range(b_loc) if ("sm" not in skip and "p2" not in skip) else []:
                att_s = sm.tile([128, 128], F32R, tag="att_s", name=f"att_s{b}")
                nc.vector.tensor_scalar_mul(out=att_s[:],
                                            in0=att_rs[b][:].bitcast(F32),
                                            scalar1=s_vec[:, 0:1])
                attTs_ps = pp_s.tile([128, 128], F32R, tag="attT_ps", name=f"attTs_ps{b}")
                nc.tensor.transpose(attTs_ps[:], att_s[:], ident_r[:])
                nc.vector.tensor_copy(attT_s[b][:], attTs_ps[:].bitcast(F32))

            # ---------------- phase 2: out = s*(att@q) + t + q ----------------
            for b in range(b_loc) if "p2" not in skip else []:
                mps_list = {}
                y_cur = None
                for c in range(n_chunks2):
                    cs = slice(c * 512, (c + 1) * 512)
                    mps = pp_m.tile([128, 512], F32, tag="mps", name="mps")
                    nc.tensor.matmul(mps[:], lhsT=attT_s[b][:],
                                     rhs=q_t[b][:, cs],
                                     start=True, stop=True)
                    mps_list[c] = mps
                    if c % 4 == 0:
                        y_cur = py.tile([128, 2048], F32, tag="y", name="y")
                    cp = c - 1
                    if cp >= 0:
                        _emit_stt(nc, mybir, y_cur if cp % 4 != 3 else y_prev,
                                  cp, mps_list.pop(cp), t_vec, q_t[b], outl, b)
                    y_prev = y_cur
                cp = n_chunks2 - 1
                _emit_stt(nc, mybir, y_cur, cp, mps_list.pop(cp), t_vec,
                          q_t[b], outl, b)

    nc.compile()
    return nc


_CACHE = {}


def _get_nc(hw=HW_FULL):
    if hw not in _CACHE:
        _CACHE[hw] = build(hw=hw,
                           use_collective=not os.environ.get("KERNEL_NO_CC"))
    return _CACHE[hw]


def kernel(x, gamma, bn_weight, bn_bias):
    x = np.ascontiguousarray(np.asarray(x, dtype=np.float32))
    gamma = np.asarray(gamma, dtype=np.float32).reshape(1, 1)
    bnw = np.ascontiguousarray(np.asarray(bn_weight, dtype=np.float32).reshape(C, 1))
    bnb = np.ascontiguousarray(np.asarray(bn_bias, dtype=np.float32).reshape(C, 1))
    Bf, Cf, Hf, Wf = x.shape
    hw = Hf * Wf
    xr = x.reshape(Bf, Cf, hw)

    nc = _get_nc(hw)
    in_maps = []
    for i in range(N_CORES):
        in_maps.append({
            "xl": np.ascontiguousarray(xr[i * B_LOC:(i + 1) * B_LOC]),
            "gamma": gamma, "bnw": bnw, "bnb": bnb,
        })
    res = run_bass_kernel_spmd(nc, in_maps, core_ids=list(range(N_CORES)))
    out = np.concatenate([r["out"] for r in res.results], axis=0)
    return out.reshape(Bf, Cf, Hf, Wf).astype(np.float32)


if __name__ == "__main__":
    rng = np.random.default_rng(0)
    x = rng.standard_normal((B, C, H, W), dtype=np.float32)
    g = rng.standard_normal((1,), dtype=np.float32)
    w = rng.random((C,), dtype=np.float32)
    bchan = rng.standard_normal((C,), dtype=np.float32)
    out = kernel(x, g, w, bchan)
    print("kernel ran, out shape", out.shape)



# revision 8
# speedup vs baseline: 1.2551x; 1.2551x over previous
"""CAM module (DANet channel attention) Trainium2 kernel, v2.

Full inputs -> host prepends fp16 copies of x in TWO layouts:
  xc : [B, C, HW]  channel-major fp16   (phase-2 rhs + residual)
  qt : [B, 4, 128, 32*130] fp16         (n-major "transposed" strips for the
       energy matmul; each 130-col chunk = 128 qT cols + ones col + pad col,
       so the PE needs NO on-chip transposes and the ones column yields qsum)
Batch is sharded over 8 cores (2 batches/core).  Energy(+qsum) accumulate in
PSUM from fp16 operands; softmax(min-e); algebraic BN stats
  S1[c] = sum_n M[c,n]   = (att @ qsum)[c]
  S2[c] = sum_n M[c,n]^2 = rowsum((att @ E) * att)
are AllReduced across cores while the channel-major reads still stream, so
exact global BN costs no extra time.  Phase 2 folds s=gamma*bnw*rsd into the
(fp16) attention, computes out = (att_s@q + t) + q fused on DVE and writes
fp16; the host upcasts to fp32.

Measured end-to-end relative error ~1.4e-3 (fp16 rounding; exact BN).
"""
import os
import sys

if '/opt/trn_rl_repo' not in sys.path:
    sys.path.insert(0, '/opt/trn_rl_repo')

import numpy as np

import concourse.bass as bass
import concourse.bacc as bacc
import concourse.mybir as mybir
import concourse.tile as tile
from concourse import masks
from concourse.bass_utils import run_bass_kernel_spmd

F32 = mybir.dt.float32
F32R = mybir.dt.float32r
F16 = mybir.dt.float16

N_CORES = 8
B, C, H, W = 16, 128, 128, 128
HW_FULL = H * W
B_LOC = B // N_CORES
BN_EPS = 1e-5

CHW = 130                   # chunk width in qt strips: 128 qT + ones + pad
N_STRIPS = 4                # qt strips per batch
SCH = 32                    # chunks per strip  (N_STRIPS*SCH*128 == HW)
STRIP_W = SCH * CHW         # 4160 fp16 elems = 8320 B per partition


def build(hw=HW_FULL, n_cores=N_CORES, b_loc=B_LOC, use_collective=True,
          loop_n=None, cc_emulate=0, skip=()):
    """skip flags (perf diagnostics only — output is garbage with any set):
    "mm" energy matmuls, "sm" softmax/stats, "p2c" phase-2 compute
    (writes still emitted, junk data), "p2" phase-2 incl. writes."""
    nc = bacc.Bacc("TRN2", target_bir_lowering=False, debug=False,
                   num_devices=n_cores)

    n_strips = max(1, hw // (SCH * 128))
    qt = nc.dram_tensor("qt", [b_loc, n_strips, 128, STRIP_W], F16,
                        kind="ExternalInput")
    xc = nc.dram_tensor("xc", [b_loc, C, hw], F16, kind="ExternalInput")
    gamma = nc.dram_tensor("gamma", [1, 1], F32, kind="ExternalInput")
    bnw = nc.dram_tensor("bnw", [C, 1], F32, kind="ExternalInput")
    bnb = nc.dram_tensor("bnb", [C, 1], F32, kind="ExternalInput")
    outl = nc.dram_tensor("out", [b_loc, C, hw], F16, kind="ExternalOutput")

    n_chunks2 = hw // 512          # phase-2 chunks
    n_qc = hw // 4096              # channel-major load strips
    n_total = float(n_cores * b_loc * hw)

    with tile.TileContext(nc) as tc:
        import contextlib
        ctx = contextlib.ExitStack()
        with ctx:
            singles = ctx.enter_context(tc.tile_pool(name="singles", bufs=1))
            pq = ctx.enter_context(tc.tile_pool(name="pq", bufs=3))
            pp_e = ctx.enter_context(tc.tile_pool(name="pp_e", bufs=1, space="PSUM"))
            pp_s = ctx.enter_context(tc.tile_pool(name="pp_s", bufs=1, space="PSUM"))
            pp_m = ctx.enter_context(tc.tile_pool(name="pp_m", bufs=3, space="PSUM"))
            sm = ctx.enter_context(tc.tile_pool(name="sm", bufs=2))
            py = ctx.enter_context(tc.tile_pool(name="py", bufs=3))
            dram = ctx.enter_context(tc.tile_pool(name="dram", bufs=1, space="DRAM"))

            # ---------------- setup (outside timing loop) ----------------
            ident = singles.tile([128, 128], F32)
            masks.make_identity(nc, ident[:])
            ident_r = singles.tile([128, 128], F32R)
            nc.vector.tensor_copy(ident_r[:], ident[:])
            ident_h = singles.tile([128, 128], F16)
            nc.vector.tensor_copy(ident_h[:], ident[:])

            bnw_sb = singles.tile([128, 1], F32)
            bnb_sb = singles.tile([128, 1], F32)
            nc.sync.dma_start(out=bnw_sb[:], in_=bnw[:, :])
            nc.sync.dma_start(out=bnb_sb[:], in_=bnb[:, :])
            gamma_sb = singles.tile([128, 1], F32)
            g_bcast = bass.AP(tensor=gamma.ap().tensor, offset=0,
                              ap=[[0, 128], [1, 1]])
            nc.sync.dma_start(out=gamma_sb[:], in_=g_bcast)
            eps_sb = singles.tile([128, 1], F32)
            nc.vector.memset(eps_sb[:], BN_EPS)

            qc = [singles.tile([128, hw], F16, name=f"qc{b}", tag=f"qc{b}")
                  for b in range(b_loc)]
            att16 = [singles.tile([128, 128], F16, name=f"att16_{b}",
                                  tag=f"att16_{b}") for b in range(b_loc)]
            attTs = [singles.tile([128, 128], F16, name=f"attTs{b}",
                                  tag=f"attTs{b}") for b in range(b_loc)]
            stats_b = [singles.tile([128, 2], F32, name=f"stats{b}",
                                    tag=f"stats{b}") for b in range(b_loc)]
            stats_tot = singles.tile([128, 2], F32)
            stats_g = singles.tile([128, 2], F32)
            meanM = singles.tile([128, 1], F32)
            em2 = singles.tile([128, 1], F32)
            varM = singles.tile([128, 1], F32)
            sd = singles.tile([128, 1], F32)
            rsd = singles.tile([128, 1], F32)
            s_vec = singles.tile([128, 1], F32)
            t_vec = singles.tile([128, 1], F32)
            if use_collective or cc_emulate:
                cc_in = dram.tile([128, 2], F32)
                cc_out = dram.tile([128, 2], F32)
            cc_hops = [singles.tile([128, 2], F32, name=f"cch{i}",
                                    tag=f"cch{i}") for i in range(cc_emulate)]

            def body():
                # ---- phase 1: energy + qsum straight off the qt stream ----
                epsum = {}
                for b in range(b_loc):
                    epsum[b] = pp_e.tile([128, CHW], F32, tag=f"epsum{b}",
                                         name=f"epsum{b}")
                    for s in range(n_strips):
                        qs = pq.tile([128, STRIP_W], F16, tag="qstrip",
                                     name="qstrip")
                        nc.sync.dma_start(out=qs[:], in_=qt[b, s, :, :])
                        for k in range(SCH) if "mm" not in skip else []:
                            c0 = k * CHW
                            nc.tensor.matmul(
                                epsum[b][:],
                                lhsT=qs[:, c0:c0 + 128],
                                rhs=qs[:, c0:c0 + CHW],
                                start=(s == 0 and k == 0),
                                stop=(s == n_strips - 1 and k == SCH - 1))

                # ---- channel-major loads (queue right behind qt reads) ----
                for b in range(b_loc):
                    for s in range(n_qc):
                        nc.sync.dma_start(
                            out=qc[b][:, s * 4096:(s + 1) * 4096],
                            in_=xc[b, :, s * 4096:(s + 1) * 4096])

                # ---- softmax + stats per batch ----
                for b in range(b_loc) if "sm" not in skip else []:
                    enr = sm.tile([128, CHW], F32, tag="enr")
                    nc.vector.tensor_copy(enr[:], epsum[b][:])
                    enr_r = sm.tile([128, CHW], F32R, tag="enr_r")
                    nc.vector.tensor_copy(enr_r[:], epsum[b][:])
                    mrow = sm.tile([128, 1], F32, tag="mrow")
                    nc.vector.tensor_reduce(out=mrow[:], in_=enr[:, 0:128],
                                            axis=mybir.AxisListType.X,
                                            op=mybir.AluOpType.min)
                    p_sb = sm.tile([128, 128], F32, tag="p_sb")
                    rs = sm.tile([128, 1], F32, tag="rs")
                    nc.scalar.activation(out=p_sb[:], in_=enr[:, 0:128],
                                         func=mybir.ActivationFunctionType.Exp,
                                         bias=mrow[:, 0:1], scale=-1.0,
                                         accum_out=rs[:, 0:1])
                    rinv = sm.tile([128, 1], F32, tag="rinv")
                    nc.vector.reciprocal(rinv[:], rs[:])
                    nc.vector.tensor_scalar_mul(out=att16[b][:], in0=p_sb[:],
                                                scalar1=rinv[:, 0:1])
                    att_f = sm.tile([128, 128], F32R, tag="att_f")
                    nc.vector.tensor_copy(att_f[:], att16[b][:])
                    attT_ps = pp_s.tile([128, 128], F32R, tag="attT_ps")
                    nc.tensor.transpose(attT_ps[:], att_f[:], ident_r[:])
                    attT_r = sm.tile([128, 128], F32R, tag="attT_r")
                    nc.vector.tensor_copy(attT_r[:], attT_ps[:].bitcast(F32))
                    r_ps = pp_s.tile([128, CHW], F32, tag="r_ps")
                    nc.tensor.matmul(r_ps[:], lhsT=attT_r[:], rhs=enr_r[:],
                                     start=True, stop=True)
                    scr = sm.tile([128, 128], F32, tag="scr")
                    nc.vector.tensor_mul(scr[:], r_ps[:, 0:128],
                                         att_f[:].bitcast(F32))
                    nc.vector.tensor_reduce(out=stats_b[b][:, 1:2], in_=scr[:],
                                            axis=mybir.AxisListType.X,
                                            op=mybir.AluOpType.add)
                    nc.vector.tensor_copy(stats_b[b][:, 0:1],
                                          r_ps[:, 128:129])

                # ---- global stats (AllReduce hidden behind xc reads) ----
                if "sm" in skip:
                    for b in range(b_loc):
                        nc.vector.memset(stats_b[b][:], 1.0)
                nc.vector.tensor_add(stats_tot[:], stats_b[0][:], stats_b[1][:]) \
                    if b_loc == 2 else \
                    nc.vector.tensor_copy(stats_tot[:], stats_b[0][:])
                if use_collective:
                    nc.gpsimd.dma_start(out=cc_in[:], in_=stats_tot[:])
                    nc.gpsimd.collective_compute(
                        "AllReduce", mybir.AluOpType.add,
                        replica_groups=[list(range(n_cores))],
                        ins=[cc_in.opt()], outs=[cc_out.opt()])
                    nc.gpsimd.dma_start(out=stats_g[:], in_=cc_out[:])
                    sg = stats_g
                    inv_n = 1.0 / n_total
                elif cc_emulate:
                    # perf-only stand-in for the AllReduce: a serial chain of
                    # DRAM round-trips on the stats path with latency >= the
                    # real collective's, to verify/charge its hiding window
                    src = stats_tot
                    for i in range(cc_emulate):
                        nc.gpsimd.dma_start(out=cc_in[:], in_=src[:])
                        nc.gpsimd.dma_start(out=cc_hops[i][:], in_=cc_in[:])
                        src = cc_hops[i]
                    nc.vector.tensor_copy(stats_g[:], src[:])
                    sg = stats_g
                    inv_n = 1.0 / (b_loc * hw)
                else:
                    sg = stats_tot
                    inv_n = 1.0 / (b_loc * hw)

                nc.vector.tensor_scalar_mul(out=meanM[:], in0=sg[:, 0:1],
                                            scalar1=inv_n)
                nc.vector.tensor_scalar_mul(out=em2[:], in0=sg[:, 1:2],
                                            scalar1=inv_n)
                nc.vector.tensor_mul(varM[:], meanM[:], meanM[:])
                nc.vector.tensor_sub(varM[:], em2[:], varM[:])
                nc.vector.tensor_mul(varM[:], varM[:], gamma_sb[:])
                nc.vector.tensor_mul(varM[:], varM[:], gamma_sb[:])
                nc.scalar.activation(out=sd[:], in_=varM[:],
                                     func=mybir.ActivationFunctionType.Sqrt,
                                     bias=eps_sb[:, 0:1], scale=1.0)
                nc.vector.reciprocal(rsd[:], sd[:])
                nc.vector.tensor_mul(s_vec[:], gamma_sb[:], bnw_sb[:])
                nc.vector.tensor_mul(s_vec[:], s_vec[:], rsd[:])
                nc.vector.tensor_mul(t_vec[:], meanM[:], s_vec[:])
                nc.vector.tensor_sub(t_vec[:], bnb_sb[:], t_vec[:])

                # fold s into attention (fp16), re-transpose
                for b in range(b_loc) if not ({"p2"} & set(skip)) else []:
                    att_s = sm.tile([128, 128], F16, tag="att_s")
                    nc.vector.tensor_scalar_mul(out=att_s[:], in0=att16[b][:],
                                                scalar1=s_vec[:, 0:1])
                    attTs_ps = pp_s.tile([128, 128], F16, tag="attTs_ps")
                    nc.tensor.transpose(attTs_ps[:], att_s[:], ident_h[:])
                    nc.vector.tensor_copy(attTs[b][:], attTs_ps[:])

                # ---- phase 2: out = (att_s @ q + t) + q, fp16 writes ----
                for b in range(b_loc) if "p2" not in skip else []:
                    y_cur = None
                    for c in range(n_chunks2):
                        cs = slice(c * 512, (c + 1) * 512)
                        if c % 4 == 0:
                            y_cur = py.tile([128, 2048], F16, tag="y",
                                            name="y")
                        if "p2c" not in skip:
                            mps = pp_m.tile([128, 512], F32, tag="mps",
                                            name="mps")
                            nc.tensor.matmul(mps[:], lhsT=attTs[b][:],
                                             rhs=qc[b][:, cs],
                                             start=True, stop=True)
                            qcol = slice((c % 4) * 512, (c % 4 + 1) * 512)
                            nc.vector.scalar_tensor_tensor(
                                out=y_cur[:, qcol], in0=mps[:],
                                scalar=t_vec[:, 0:1], in1=qc[b][:, cs],
                                op0=mybir.AluOpType.add, op1=mybir.AluOpType.add)
                        if c % 4 == 3:
                            g0 = (c - 3) * 512
                            nc.sync.dma_start(out=outl[b, :, g0:g0 + 2048],
                                              in_=y_cur[:])

            if loop_n is not None:
                with tc.For_i(0, loop_n) as _:
                    body()
            else:
                body()

    nc.compile()
    return nc


_CACHE = {}


def _get_nc(hw=HW_FULL):
    key = hw
    if key not in _CACHE:
        _CACHE[key] = build(hw=hw,
                            use_collective=not os.environ.get("KERNEL_NO_CC"))
    return _CACHE[key]


def _prep_inputs(x, gamma, bn_weight, bn_bias):
    """Host-side prep: fp16 copies of x in channel-major and transposed
    (strip) layouts, per-core shards."""
    x = np.asarray(x, dtype=np.float32)
    Bf, Cf, Hf, Wf = x.shape
    hw = Hf * Wf
    xr = x.reshape(Bf, Cf, hw)
    xc16 = xr.astype(np.float16)

    n_strips = hw // (SCH * 128)
    # qt[b, s, p, k*130 + c] = x[b, c, s*4096 + k*128 + p]  (+ ones col)
    qtp = np.empty((Bf, n_strips, 128, SCH, CHW), np.float16)
    src = xr.reshape(Bf, Cf, n_strips, SCH, 128)          # [b, c, s, k, p]
    qtp[..., :128] = src.transpose(0, 2, 4, 3, 1).astype(np.float16)
    qtp[..., 128] = 1.0
    qtp[..., 129] = 0.0
    qtp = qtp.reshape(Bf, n_strips, 128, STRIP_W)

    gamma = np.asarray(gamma, dtype=np.float32).reshape(1, 1)
    bnw = np.ascontiguousarray(
        np.asarray(bn_weight, dtype=np.float32).reshape(Cf, 1))
    bnb = np.ascontiguousarray(
        np.asarray(bn_bias, dtype=np.float32).reshape(Cf, 1))

    in_maps = []
    for i in range(N_CORES):
        bs = slice(i * B_LOC, (i + 1) * B_LOC)
        in_maps.append({
            "qt": np.ascontiguousarray(qtp[bs]),
            "xc": np.ascontiguousarray(xc16[bs]),
            "gamma": gamma, "bnw": bnw, "bnb": bnb,
        })
    return in_maps, hw


def kernel(x, gamma, bn_weight, bn_bias):
    x = np.asarray(x, dtype=np.float32)
    Bf, Cf, Hf, Wf = x.shape
    in_maps, hw = _prep_inputs(x, gamma, bn_weight, bn_bias)
    nc = _get_nc(hw)
    res = run_bass_kernel_spmd(nc, in_maps, core_ids=list(range(N_CORES)))
    out = np.concatenate([np.asarray(r["out"]) for r in res.results], axis=0)
    return out.astype(np.float32).reshape(Bf, Cf, Hf, Wf)


if __name__ == "__main__":
    rng = np.random.default_rng(0)
    x = rng.standard_normal((B, C, H, W), dtype=np.float32)
    g = rng.standard_normal((1,), dtype=np.float32)
    w = rng.random((C,), dtype=np.float32)
    bchan = rng.standard_normal((C,), dtype=np.float32)
    out = kernel(x, g, w, bchan)
    print("kernel ran, out shape", out.shape, out.dtype)
